# revision 28
# baseline (speedup 1.0000x reference)
"""Trainium2 Bass kernel for nn_ColorNet: 7x7 box conv s2 -> 3x3 maxpool s2 ->
27 sequential 3x3 box convs (strides [1]*6+[2]+[1]*8+[2]+[1]*11).

Decomposition (exact linear algebra):
  - every 2D stage is separable: stage(X) = Bv @ X @ Bh^T
  - conv1:  Y1 = B1 @ X @ B1^T  with B1 = 256x512 banded 7-tap stride-2 matrix
  - maxpool: separable 3-window stride-2 max along each axis
  - the 27-conv tail is linear: collapses to  F = M @ Z @ M^T, M = 32x128

Mapping (per core, 16 images, data parallel across 8 cores), all bf16
(x converted to bf16 on the host - halves HBM traffic; error gate is 2e-2):
  C_v : stationary = X blocks [h,w] (16 LDW), moving = banded B1v^T spans
        (~66 cols per h-chunk) -> U^T = X^T B1v^T [w part, h' free]
        (no input-side transposes needed)
  evac: U^T psum -> sbuf bf16, split DVE/ACT
  C_h : stationary = U^T blocks (8 LDW), moving = B1h^T spans
        -> V [h' part, w' free]
  M_w : ACT packs V's odd w' cols to sbuf (TensorTensor reads at most one
        PSUM operand), DVE pairwise + shifted max
  T2  : 2 regular matmuls vs identity (not transpose-mode: that skips the
        HAM fast clock; DMA transpose measured far worse on HW)
  M_h : same pack+max pattern -> Z^T [w'' part, h'' free]
  tail: W1 = Z Mh^T via lhsT=Z^T, then F = Mv W1 via lhsT=wsegv (F direct)
Emission is software-pipelined: image n's C_v/evac are emitted before image
n-1's C_h..tail so the in-order PE queue never idles on the evacuation.
Per-image PSUM tiles are small and short-lived (utp 2 banks x2, vp 1x2,
ztp/wfp 1x1 each) - every consolidation attempt measured slower on HW.
"""
import numpy as np

N_CORES = 8
N_IMGS = 128
PER_CORE = N_IMGS // N_CORES  # 16

_STRIDES_3x3 = [1] * 6 + [2] + [1] * 8 + [2] + [1] * 11


def _conv_matrix(n_in, taps, s, p, dtype=np.float64):
    k = len(taps)
    n_out = (n_in + 2 * p - k) // s + 1
    A = np.zeros((n_out, n_in), dtype=dtype)
    for i in range(n_out):
        for j in range(k):
            idx = s * i + j - p
            if 0 <= idx < n_in:
                A[i, idx] = taps[j]
    return A


def _rank1_taps(f2d):
    """Split a (separable) 2D kernel into vertical/horizontal 1D taps."""
    f = np.asarray(f2d, dtype=np.float64)
    u, s, vt = np.linalg.svd(f)
    assert s[0] > 0 and (len(s) == 1 or s[1] < 1e-6 * s[0]), "filter not rank-1"
    kv = u[:, 0] * np.sqrt(s[0])
    kh = vt[0, :] * np.sqrt(s[0])
    if kv.sum() < 0:  # fix sign convention
        kv, kh = -kv, -kh
    return kv, kh


# nonzero h' spans of B1v^T per 128-row h-chunk, split into a disjoint "main"
# span (overwrites fresh PSUM) plus a 3-col "fix" span that accumulates into
# the previous chunk's already-written region.  Disjoint mains keep every
# matmul's touched PSUM range uniformly fresh-or-written (HW has_written
# semantics and the simulator's coarse pending-zero model both need that).
_MAIN_SPANS = [(0, 66), (66, 64), (130, 64), (194, 62)]
_FIX_SPANS = [None, (63, 3), (127, 3), (191, 3)]


def _host_matrices(filter1, filter2):
    """Build the constant matrices shipped to every core (bf16)."""
    import ml_dtypes
    bf16 = ml_dtypes.bfloat16

    kv1, kh1 = _rank1_taps(filter1[0, 0])
    kv2, kh2 = _rank1_taps(filter2[0, 0])
    B1v = _conv_matrix(512, kv1, 2, 3)
    B1h = _conv_matrix(512, kh1, 2, 3)

    def tail(taps):
        n = 128
        M = np.eye(n)
        for s in _STRIDES_3x3:
            A = _conv_matrix(n, taps, s, 1)
            M = A @ M
            n = A.shape[0]
        return M  # 32x128

    Mv = tail(kv2)
    Mh = tail(kh2)

    def spans_pack(B):
        # pack B^T's per-chunk main+fix column spans: [128, sum(ln)]
        nsp = sum(ln for lo, ln in _MAIN_SPANS) + sum(
            ln for s in _FIX_SPANS if s for lo, ln in [s])
        packed = np.zeros((128, nsp))
        off = 0
        cover = np.zeros_like(B)
        for hc in range(4):
            for span in (_MAIN_SPANS[hc], _FIX_SPANS[hc]):
                if span is None:
                    continue
                lo, ln = span
                blk = B[lo:lo + ln, 128 * hc:128 * (hc + 1)]  # [ln, 128]
                packed[:, off:off + ln] = blk.T
                cover[lo:lo + ln, 128 * hc:128 * (hc + 1)] = 1
                off += ln
        assert not np.any((B != 0) & (cover == 0)), "spans miss nonzeros"
        return packed

    consts = {
        "wv": spans_pack(B1v).astype(bf16),
        "wh": spans_pack(B1h).astype(bf16),
        "wsegv": Mv.T.astype(bf16).copy(),   # [128, 32] = Mv^T
        "wsegh": Mh.T.astype(bf16).copy(),   # [128, 32] = Mh^T
        "id128": np.eye(128, dtype=bf16),
    }
    return consts


_NC_CACHE = {}


def _build_nc(reps=1, stage=7, bufs_utp=2, bufs_vp=2):
    key = (reps, stage, bufs_utp, bufs_vp)
    if key in _NC_CACHE:
        return _NC_CACHE[key]
    import concourse.bass as bass
    import concourse.tile as tile
    from concourse import bacc, mybir

    f32 = mybir.dt.float32
    bf16 = mybir.dt.bfloat16
    NSP = sum(ln for _, ln in _MAIN_SPANS) + sum(
        ln for s in _FIX_SPANS if s for _, ln in [s])

    nc = bacc.Bacc("TRN2", target_bir_lowering=False, debug=False,
                   num_devices=N_CORES)
    x_d = nc.dram_tensor("x", [PER_CORE, 1, 512, 512], bf16,
                         kind="ExternalInput").ap()
    wv_d = nc.dram_tensor("wv", [128, NSP], bf16, kind="ExternalInput").ap()
    wh_d = nc.dram_tensor("wh", [128, NSP], bf16, kind="ExternalInput").ap()
    wsegv_d = nc.dram_tensor("wsegv", [128, 32], bf16,
                             kind="ExternalInput").ap()
    wsegh_d = nc.dram_tensor("wsegh", [128, 32], bf16,
                             kind="ExternalInput").ap()
    y_d = nc.dram_tensor("y", [PER_CORE, 1, 32, 32], f32,
                         kind="ExternalOutput").ap()

    with tile.TileContext(nc) as tc:
        with (
            tc.tile_pool(name="consts", bufs=1) as cpool,
            tc.tile_pool(name="x", bufs=4) as xpool,
            tc.tile_pool(name="ut", bufs=3) as utpool,
            tc.tile_pool(name="pw", bufs=2) as pwpool,
            tc.tile_pool(name="small", bufs=2) as smpool,
            tc.tile_pool(name="utp", bufs=1, space="PSUM") as utppool,
            tc.tile_pool(name="zp", bufs=1, space="PSUM") as zppool,
        ):
            # --- load constants once ---
            wv = cpool.tile([128, NSP], bf16, tag="wv")
            nc.sync.dma_start(wv[:], wv_d)
            wh = cpool.tile([128, NSP], bf16, tag="wh")
            nc.sync.dma_start(wh[:], wh_d)
            wsegv = cpool.tile([128, 32], bf16, tag="wsegv")
            nc.sync.dma_start(wsegv[:], wsegv_d)
            wsegh = cpool.tile([128, 32], bf16, tag="wsegh")
            nc.sync.dma_start(wsegh[:], wsegh_d)

            outs_all = smpool.tile([32, 32 * PER_CORE], f32, tag="outsall")
            if stage < 7:
                nc.vector.memset(outs_all[:], 0.0)
            import contextlib
            loop_cm = (tc.For_i(0, reps, 1) if reps > 1
                       else contextlib.nullcontext())
            with loop_cm:
              # Software-pipelined emission: image n's C_v/evac are emitted
              # BEFORE image n-1's C_h..tail, so the in-order PE queue fills
              # the evac wait of image n-1 with image n's matmuls.
              rest_q = []

              def emit_front(n, xt):
                # ---- C_v: U^T[wb] += X[hc,wb]^T @ B1v^T[hc, span] ----
                utp = utppool.tile([128, 1024], f32, tag="utp")
                for wb in range(4):
                    out = utp[:, 256 * wb:256 * (wb + 1)]
                    off = 0
                    for hc in range(4):
                        lhsT = xt[:, 512 * hc + 128 * wb:
                                  512 * hc + 128 * (wb + 1)]
                        for span in (_MAIN_SPANS[hc], _FIX_SPANS[hc]):
                            if span is None:
                                continue
                            lo, ln = span
                            nc.tensor.matmul(
                                out[:, lo:lo + ln], lhsT,
                                wv[:, off:off + ln],
                                start=(off == 0), stop=(off + ln == NSP))
                            off += ln
                if stage < 3:
                    return None
                # ---- U^T evac: psum -> sbuf bf16 (split DVE/ACT) ----
                ut = utpool.tile([128, 1024], bf16, tag="ut")
                nc.vector.tensor_copy(ut[:, 0:512], utp[:, 0:512])
                nc.scalar.copy(ut[:, 512:1024], utp[:, 512:1024])
                return ut

              def emit_rest(n, ut):
                if stage < 4 or ut is None:
                    return
                # ---- C_h: V[hb] += U^T[wc,hb]^T @ B1h^T[wc, span] ----
                vpt = vppool.tile([128, 512], f32, tag="vp")
                vp = vpt[:]
                for hb in range(2):
                    out = vp[:, 256 * hb:256 * (hb + 1)]
                    off = 0
                    for wc in range(4):
                        lhsT = ut[:, 256 * wc + 128 * hb:
                                  256 * wc + 128 * (hb + 1)]
                        for span in (_MAIN_SPANS[wc], _FIX_SPANS[wc]):
                            if span is None:
                                continue
                            lo, ln = span
                            nc.tensor.matmul(
                                out[:, lo:lo + ln], lhsT,
                                wh[:, off:off + ln],
                                start=(off == 0), stop=(off + ln == NSP))
                            off += ln

                if stage < 5:
                    return
                # ---- M_w: ACT packs odd cols to sbuf, DVE maxes ----
                vv = vp.rearrange("p (b j k) -> p b j k", b=2, k=2)
                vodd = pwpool.tile([128, 256], bf16, tag="vodd")
                voddv = vodd[:].rearrange("p (b j) -> p b j", b=2)
                nc.scalar.copy(voddv, vv[:, :, :, 1])
                pwt = pwpool.tile([128, 256], bf16, tag="pw")
                pw = pwt[:]
                pwv = pw.rearrange("p (b j) -> p b j", b=2)
                nc.vector.tensor_max(pwv, vv[:, :, :, 0], voddv)
                nc.vector.tensor_max(pwv[:, :, 1:128], pwv[:, :, 1:128],
                                     voddv[:, :, 0:127])

                if stage < 6:
                    return
                # ---- T2 as 2 regular matmuls vs identity (stays at the
                # warm PE clock; transpose-mode doesn't engage HAM) ----
                ztpt = zppool.tile([128, 256], f32, tag="ztp")
                ztp = ztpt[:]
                for hb in range(2):
                    nc.tensor.matmul(ztp[:, 128 * hb:128 * (hb + 1)],
                                     pw[:, 128 * hb:128 * (hb + 1)],
                                     id128[:], start=True, stop=True)

                # ---- M_h: ACT packs odd cols to sbuf, DVE maxes ----
                zv = ztp.rearrange("p (j k) -> p j k", k=2)
                zodd = smpool.tile([128, 128], bf16, tag="zodd")
                nc.scalar.copy(zodd[:], zv[:, :, 1])
                zt = smpool.tile([128, 128], bf16, tag="zt")
                nc.vector.tensor_max(zt[:], zv[:, :, 0], zodd[:])
                nc.vector.tensor_max(zt[:, 1:128], zt[:, 1:128],
                                     zodd[:, 0:127])

                if stage < 7:
                    return
                # ---- tail: W1 = Z @ Mh^T (lhsT=Z^T), F = Mv @ W1 ----
                wfpt = zppool.tile([128, 64], f32, tag="wfp")
                wfp = wfpt[:]
                nc.tensor.matmul(wfp[:, 0:32], zt[:], wsegh[:],
                                 start=True, stop=True)
                w1 = smpool.tile([128, 32], bf16, tag="w1")
                nc.vector.tensor_copy(w1[:], wfp[:, 0:32])
                nc.tensor.matmul(wfp[0:32, 32:64], wsegv[:], w1[:],
                                 start=True, stop=True)
                nc.scalar.copy(outs_all[:, 32 * n:32 * (n + 1)],
                               wfp[0:32, 32:64])

              for g in range(PER_CORE // 4):
                # ---- load 4 images in one DMA: [128 p, (i, hc, w)] ----
                xt4 = xpool.tile([128, 4 * 2048], bf16, tag="xt4")
                nc.sync.dma_start(
                    xt4[:].rearrange("p (i c w) -> p i c w", i=4, c=4),
                    x_d[4 * g:4 * (g + 1), 0].rearrange(
                        "i (c p) w -> p i c w", p=128))

                for i in range(4):
                    if stage < 2:
                        continue
                    n = 4 * g + i
                    ut = emit_front(n, xt4[:, 2048 * i:2048 * (i + 1)])
                    if rest_q:
                        emit_rest(*rest_q.pop())
                    rest_q.append((n, ut))
              while rest_q:
                emit_rest(*rest_q.pop())
              nc.sync.dma_start(
                  y_d[:, 0].rearrange("n h w -> h n w"),
                  outs_all[:].rearrange("h (n w) -> h n w", w=32))

    nc.compile()
    _NC_CACHE[key] = nc
    return nc


def _in_maps(x, filter1, filter2):
    import ml_dtypes
    x = np.asarray(x, dtype=np.float32)
    assert x.shape == (N_IMGS, 1, 512, 512)
    xb = np.ascontiguousarray(x.astype(ml_dtypes.bfloat16))
    consts = _host_matrices(np.asarray(filter1), np.asarray(filter2))
    in_maps = []
    for c in range(N_CORES):
        m = {"x": xb[c * PER_CORE:(c + 1) * PER_CORE]}
        m.update(consts)
        in_maps.append(m)
    return in_maps


def kernel(x, filter1, filter2):
    from concourse.bass_utils import run_bass_kernel_spmd

    in_maps = _in_maps(x, filter1, filter2)
    nc = _build_nc()
    res = run_bass_kernel_spmd(nc, in_maps, list(range(N_CORES)))
    y = np.concatenate([res.results[c]["y"] for c in range(N_CORES)], axis=0)
    return y.astype(np.float32)
